# revision 3
# baseline (speedup 1.0000x reference)
"""CAM-module (complex channel-attention) Bass kernel for Trainium2.

Problem: x (2, 8, 512, 4, 32, 32) fp32 -> out same shape.
  qr, qi = x[0].reshape(B,C,N), x[1].reshape(B,C,N)   C=512, N=4096
  er = qr qr^T - qi qi^T ; ei = qr qi^T + qi qr^T     (B, C, C)
  F  = (rowmax(er)-er)^2 + (rowmax(ei)-ei)^2
  att = softmax_row(F)                                 (ultra-sharp)
  out = stack(g*att@qr + qr_in, g*att@qi + qi_in)

Sharding: data-parallel over batch B across 8 NeuronCores (core b = batch b).

Numerics: the softmax is near-one-hot with top-2 F-gaps as small as ~18, so
the Gram phase needs fp32-quality precision. bf16 matmuls alone flip argmax
rows. We use a bf16 hi/lo split (A = bf16(q), Bl = bf16(q - A)) and compute
  qq'^T ~= A A'^T + A Bl'^T + Bl A'^T   (dropping Bl Bl'^T, err ~1e-3)
at full bf16 PE rate. Symmetric cross terms are computed once and
symmetrized via PE transposes:
  er = AA_r - AA_i + M12 + M12^T,  M12 = A_r Bl_r^T - A_i Bl_i^T
  ei = S + S^T,                    S   = A_r A_i^T + A_r Bl_i^T + Bl_r A_i^T
Phase 2 (att @ q) tolerates bf16 (verified: absmax ~7e-4 on output).
"""
import sys, os
sys.path.insert(0, '/opt/trn_rl_repo')

import numpy as np
from contextlib import ExitStack

import concourse.bass as bass
import concourse.mybir as mybir
import concourse.tile as tile
from concourse import bacc
from concourse.bass_utils import run_bass_kernel_spmd
from concourse.masks import make_identity

F32 = mybir.dt.float32
BF16 = mybir.dt.bfloat16
AX = mybir.AxisListType
AF = mybir.ActivationFunctionType
OP = mybir.AluOpType

C = 512          # channels
N = 4096         # spatial (4*32*32)
NK = N // 128    # 32 n-chunks (contraction)
MC = C // 128    # 4 c-chunks


def build_kernel():
    nc = bacc.Bacc("TRN2", target_bir_lowering=False, debug=False,
                   enable_asserts=False)
    x_dram = nc.dram_tensor("x", (2, C, N), F32, kind="ExternalInput").ap()
    g_dram = nc.dram_tensor("gamma", (1,), F32, kind="ExternalInput").ap()
    y_dram = nc.dram_tensor("y", (2, C, N), F32, kind="ExternalOutput").ap()

    with tile.TileContext(nc) as tc, ExitStack() as ctx:
        const = ctx.enter_context(tc.tile_pool(name="const", bufs=1))
        small = ctx.enter_context(tc.tile_pool(name="small", bufs=10))
        stage = ctx.enter_context(tc.tile_pool(name="stage", bufs=2))
        sqf = ctx.enter_context(tc.tile_pool(name="sqf", bufs=1))
        sqb = ctx.enter_context(tc.tile_pool(name="sqb", bufs=1))

        ident32 = const.tile([128, 128], F32, tag="id32")
        make_identity(nc, ident32[:])
        ident16 = const.tile([128, 128], BF16, tag="id16")
        make_identity(nc, ident16[:])
        g_bc = const.tile([128, 1], F32, tag="gbc")
        nc.sync.dma_start(g_bc[:], g_dram[None, :].partition_broadcast(128))

        # persistent [512,512] matrices as [128, 4*512] (row-chunk r at cols r*512)
        er_sb = sqf.tile([128, MC * C], F32, tag="er")
        m12_sb = sqf.tile([128, MC * C], F32, tag="m12")
        s_sb = sqf.tile([128, MC * C], F32, tag="s")
        att_sb = sqb.tile([128, MC * C], BF16, tag="att")
        attT_sb = sqb.tile([128, MC * C], BF16, tag="attT")

        with tc.tile_pool(name="opsT", bufs=4) as opsT:
            # transposed bf16 hi/lo operands, [128, NK*512]; chunk k at cols k*512
            arT = opsT.tile([128, NK * C], BF16, tag="opsT")
            brT = opsT.tile([128, NK * C], BF16, tag="opsT")
            aiT = opsT.tile([128, NK * C], BF16, tag="opsT")
            biT = opsT.tile([128, NK * C], BF16, tag="opsT")

            # ---------------- Phase 0: load, transpose, hi/lo split ----------
            with tc.tile_pool(name="tpose", bufs=3, space="PSUM") as tpose:
                for ti, (aT, bT) in enumerate([(arT, brT), (aiT, biT)]):
                    for j in range(MC):  # c-chunk
                        q_j = stage.tile([128, N], F32, tag="stage")
                        nc.sync.dma_start(q_j[:], x_dram[ti, j * 128:(j + 1) * 128, :])
                        for kq in range(NK // 4):  # groups of 4 n-chunks
                            pt = tpose.tile([128, 512], F32, tag="pt")
                            for t in range(4):
                                k = kq * 4 + t
                                nc.tensor.transpose(
                                    pt[:, t * 128:(t + 1) * 128],
                                    q_j[:, k * 128:(k + 1) * 128], ident32[:])
                            # strided dest: cols (kq*4+t)*512 + j*128, t=0..4
                            aT_v = aT[:].rearrange("p (k c) -> p k c", c=C)[
                                :, kq * 4:(kq + 1) * 4, j * 128:(j + 1) * 128]
                            bT_v = bT[:].rearrange("p (k c) -> p k c", c=C)[
                                :, kq * 4:(kq + 1) * 4, j * 128:(j + 1) * 128]
                            pt_v = pt[:].rearrange("p (t c) -> p t c", c=128)
                            nc.scalar.copy(aT_v, pt_v)            # cast f32->bf16
                            nc.vector.tensor_sub(bT_v, pt_v, aT_v)  # lo residual

            # ---------------- Phase 1: 7 Gram units, 4 m-passes -------------
            with tc.tile_pool(name="acc", bufs=8, space="PSUM") as acc:
                for m in range(MC):
                    u1 = acc.tile([128, C], F32, tag="acc")  # AA_r
                    u2 = acc.tile([128, C], F32, tag="acc")  # AA_i
                    m1 = acc.tile([128, C], F32, tag="acc")  # A_r Bl_r^T
                    m2 = acc.tile([128, C], F32, tag="acc")  # A_i Bl_i^T
                    s = acc.tile([128, C], F32, tag="acc")   # S
                    for k in range(NK):
                        st, sp = (k == 0), (k == NK - 1)
                        lo, hi = k * C, (k + 1) * C
                        ar_m = arT[:, lo + m * 128: lo + (m + 1) * 128]
                        ai_m = aiT[:, lo + m * 128: lo + (m + 1) * 128]
                        br_m = brT[:, lo + m * 128: lo + (m + 1) * 128]
                        ar_k, ai_k = arT[:, lo:hi], aiT[:, lo:hi]
                        br_k, bi_k = brT[:, lo:hi], biT[:, lo:hi]
                        nc.tensor.matmul(u1[:], ar_m, ar_k, start=st, stop=sp)
                        nc.tensor.matmul(u2[:], ai_m, ai_k, start=st, stop=sp)
                        nc.tensor.matmul(m1[:], ar_m, br_k, start=st, stop=sp)
                        nc.tensor.matmul(m2[:], ai_m, bi_k, start=st, stop=sp)
                        nc.tensor.matmul(s[:], ar_m, ai_k, start=st, stop=(False))
                        nc.tensor.matmul(s[:], ar_m, bi_k, start=False, stop=False)
                        nc.tensor.matmul(s[:], br_m, ai_k, start=False, stop=sp)
                    # evacuate row-chunk m
                    er_m = er_sb[:, m * C:(m + 1) * C]
                    nc.vector.tensor_copy(er_m, u1[:])
                    nc.vector.tensor_sub(er_m, er_m, u2[:])
                    m12_m = m12_sb[:, m * C:(m + 1) * C]
                    nc.vector.tensor_copy(m12_m, m1[:])
                    nc.vector.tensor_sub(m12_m, m12_m, m2[:])
                    nc.scalar.copy(s_sb[:, m * C:(m + 1) * C], s[:])

            # -------------- symmetrize: er += M12 + M12^T; ei = S + S^T -----
            with tc.tile_pool(name="sym", bufs=4, space="PSUM") as sym, \
                 tc.tile_pool(name="attp", bufs=2, space="PSUM") as attp:
                # er += M12 (non-transposed part)
                for m in range(MC):
                    er_m = er_sb[:, m * C:(m + 1) * C]
                    nc.vector.tensor_add(er_m, er_m, m12_sb[:, m * C:(m + 1) * C])
                # er[b] += M12[:, b-chunk]^T
                for b in range(MC):
                    pt = sym.tile([128, C], F32, tag="symt")
                    for a in range(MC):
                        nc.tensor.transpose(
                            pt[:, a * 128:(a + 1) * 128],
                            m12_sb[:, a * C + b * 128: a * C + (b + 1) * 128],
                            ident32[:])
                    er_b = er_sb[:, b * C:(b + 1) * C]
                    nc.vector.tensor_add(er_b, er_b, pt[:])
                # ei = S + S^T (in-place into s_sb; all transposes emitted first)
                spt = []
                for b in range(MC):
                    pt = sym.tile([128, C], F32, tag="symt")
                    for a in range(MC):
                        nc.tensor.transpose(
                            pt[:, a * 128:(a + 1) * 128],
                            s_sb[:, a * C + b * 128: a * C + (b + 1) * 128],
                            ident32[:])
                    spt.append(pt)
                for b in range(MC):
                    ei_b = s_sb[:, b * C:(b + 1) * C]
                    nc.vector.tensor_add(ei_b, ei_b, spt[b][:])

                # -------------- softmax over squared magnitude --------------
                for m in range(MC):
                    er_m = er_sb[:, m * C:(m + 1) * C]
                    ei_m = s_sb[:, m * C:(m + 1) * C]
                    nmx_r = small.tile([128, 1], F32, tag="small")
                    nmx_i = small.tile([128, 1], F32, tag="small")
                    nc.vector.reduce_max(nmx_r[:], er_m, axis=AX.X, negate=True)
                    nc.vector.reduce_max(nmx_i[:], ei_m, axis=AX.X, negate=True)
                    sq1 = stage.tile([128, C], F32, tag="stage")
                    nc.scalar.activation(sq1[:], er_m, AF.Square, bias=nmx_r[:, 0:1])
                    sq2 = stage.tile([128, C], F32, tag="stage")
                    nc.scalar.activation(sq2[:], ei_m, AF.Square, bias=nmx_i[:, 0:1])
                    nc.vector.tensor_add(sq1[:], sq1[:], sq2[:])  # F
                    nfm = small.tile([128, 1], F32, tag="small")
                    nc.vector.reduce_max(nfm[:], sq1[:], axis=AX.X, negate=True)
                    rsum = small.tile([128, 1], F32, tag="small")
                    nc.scalar.activation(sq2[:], sq1[:], AF.Exp,
                                         bias=nfm[:, 0:1], accum_out=rsum[:, 0:1])
                    rinv = small.tile([128, 1], F32, tag="small")
                    nc.vector.reciprocal(rinv[:], rsum[:])
                    nc.vector.tensor_scalar_mul(
                        att_sb[:, m * C:(m + 1) * C], sq2[:], rinv[:, 0:1])

                # -------------- attT (bf16 PE transposes) -------------------
                for d in range(MC):
                    pt = attp.tile([128, C], BF16, tag="attt")
                    for m in range(MC):
                        nc.tensor.transpose(
                            pt[:, m * 128:(m + 1) * 128],
                            att_sb[:, m * C + d * 128: m * C + (d + 1) * 128],
                            ident16[:])
                    nc.scalar.copy(attT_sb[:, d * C:(d + 1) * C], pt[:])

        # ---------------- Phase 2: out = gamma*(att @ q) + x ----------------
        with tc.tile_pool(name="slab32", bufs=4) as slab32, \
             tc.tile_pool(name="slab16", bufs=4) as slab16, \
             tc.tile_pool(name="ysb", bufs=6) as ysbp, \
             tc.tile_pool(name="out", bufs=4, space="PSUM") as outp:
            NJ = N // 512
            for j in range(NJ):
                slabs, slabs_h = [], []
                for ti in range(2):
                    sl = slab32.tile([128, MC, 512], F32, tag="sl32")
                    for d in range(MC):
                        nc.sync.dma_start(
                            sl[:, d, :],
                            x_dram[ti, d * 128:(d + 1) * 128, j * 512:(j + 1) * 512])
                    sh = slab16.tile([128, MC, 512], BF16, tag="sl16")
                    if ti == 0:
                        nc.scalar.copy(sh[:], sl[:])
                    else:
                        nc.vector.tensor_copy(sh[:], sl[:])
                    slabs.append(sl)
                    slabs_h.append(sh)
                for ti in range(2):
                    for m in range(MC):
                        ops = outp.tile([128, 512], F32, tag="out")
                        for d in range(MC):
                            nc.tensor.matmul(
                                ops[:],
                                attT_sb[:, d * C + m * 128: d * C + (m + 1) * 128],
                                slabs_h[ti][:, d, :],
                                start=(d == 0), stop=(d == MC - 1))
                        y_t = ysbp.tile([128, 512], F32, tag="ysb")
                        nc.vector.scalar_tensor_tensor(
                            y_t[:], ops[:], g_bc[:, 0:1], slabs[ti][:, m, :],
                            op0=OP.mult, op1=OP.add)
                        nc.sync.dma_start(
                            y_dram[ti, m * 128:(m + 1) * 128, j * 512:(j + 1) * 512],
                            y_t[:])

    nc.compile()
    return nc


_NC_CACHE = None


def kernel(x: np.ndarray, gamma: np.ndarray) -> np.ndarray:
    global _NC_CACHE
    if _NC_CACHE is None:
        _NC_CACHE = build_kernel()
    nc = _NC_CACHE
    B = x.shape[1]
    x = np.ascontiguousarray(x, dtype=np.float32)
    in_maps = [{"x": np.ascontiguousarray(x[:, b]).reshape(2, C, N),
                "gamma": np.ascontiguousarray(gamma, dtype=np.float32)}
               for b in range(B)]
    res = run_bass_kernel_spmd(nc, in_maps, core_ids=list(range(B)))
    y = np.stack([res.results[b]["y"] for b in range(B)], axis=1)
    return y.reshape(x.shape)


# revision 10
# speedup vs baseline: 75.0477x; 75.0477x over previous
"""CAM-module (complex channel-attention) Bass kernel for Trainium2.

Problem: x (2, 8, 512, 4, 32, 32) fp32 -> out same shape.
  qr, qi = x[0].reshape(B,C,N), x[1].reshape(B,C,N)   C=512, N=4096
  er = qr qr^T - qi qi^T ; ei = qr qi^T + qi qr^T     (B, C, C)
  F  = (rowmax(er)-er)^2 + (rowmax(ei)-ei)^2
  att = softmax_row(F)                                 (ultra-sharp)
  out = stack(g*att@qr + qr_in, g*att@qi + qi_in)

Sharding: data-parallel over batch B across 8 NeuronCores (core b = batch b).

Numerics: the softmax is near-one-hot with top-2 F-gaps as small as ~18, so
the Gram phase needs fp32-quality precision; single-dtype bf16/tf32 matmuls
flip argmax rows. We use an fp16 hi/lo split (A = f16(q), Bl = f16(q - A))
and compute
  qq'^T ~= A A'^T + A Bl'^T + Bl A'^T   (dropping Bl Bl'^T, err ~1e-5)
at full 1-cycle/row PE rate. Symmetric cross terms are computed once and
symmetrized via PE transposes:
  er = AA_r - AA_i + M12 + M12^T,  M12 = A_r Bl_r^T - A_i Bl_i^T
  ei = S + S^T,                    S   = A_r A_i^T + A_r Bl_i^T + Bl_r A_i^T
Phase 2 (att @ q) runs in fp16 (error ~1e-5 of O(1) outputs).

Schedule notes:
 - input loaded in column-quarters so the first Gram pass interleaves with
   the PE input transposes;
 - symmetrization transpose-adds are emitted incrementally after each
   m-pass evacuation (only the last pass's blocks land in the tail);
 - phase-2 slab DMA is emitted before the softmax tail so it overlaps.
"""
import sys, os
sys.path.insert(0, '/opt/trn_rl_repo')

import numpy as np
from contextlib import ExitStack

import concourse.bass as bass
import concourse.mybir as mybir
import concourse.tile as tile
from concourse import bacc
from concourse.bass_utils import run_bass_kernel_spmd
from concourse.masks import make_identity

F32 = mybir.dt.float32
F16 = mybir.dt.float16
AX = mybir.AxisListType
AF = mybir.ActivationFunctionType
OP = mybir.AluOpType

C = 512          # channels
N = 4096         # spatial (4*32*32)
NK = N // 128    # 32 n-chunks (contraction)
MC = C // 128    # 4 c-chunks
NQ = 4           # column quarters for load/transpose pipeline
KQ = NK // NQ    # 8 n-chunks per quarter
NJ = N // 512    # phase-2 column blocks


def _gram_mms(nc, accs, arT, brT, aiT, biT, m, k):
    """The 7 phase-1 matmuls for (m-block, k-chunk), grouped by lhsT."""
    u1, u2, m1, m2, s = accs
    st, sp = (k == 0), (k == NK - 1)
    lo = k * C
    ar_m = arT[:, lo + m * 128: lo + (m + 1) * 128]
    ai_m = aiT[:, lo + m * 128: lo + (m + 1) * 128]
    br_m = brT[:, lo + m * 128: lo + (m + 1) * 128]
    ar_k = arT[:, lo:lo + C]
    ai_k = aiT[:, lo:lo + C]
    br_k = brT[:, lo:lo + C]
    bi_k = biT[:, lo:lo + C]
    # lhsT = ar_m
    nc.tensor.matmul(u1[:], ar_m, ar_k, start=st, stop=sp)
    nc.tensor.matmul(m1[:], ar_m, br_k, start=st, stop=sp)
    nc.tensor.matmul(s[:], ar_m, ai_k, start=st, stop=False)
    nc.tensor.matmul(s[:], ar_m, bi_k, start=False, stop=False)
    # lhsT = ai_m
    nc.tensor.matmul(u2[:], ai_m, ai_k, start=st, stop=sp)
    nc.tensor.matmul(m2[:], ai_m, bi_k, start=st, stop=sp)
    # lhsT = br_m
    nc.tensor.matmul(s[:], br_m, ai_k, start=False, stop=sp)


def build_kernel():
    nc = bacc.Bacc("TRN2", target_bir_lowering=False, debug=False,
                   enable_asserts=False)
    x_dram = nc.dram_tensor("x", (2, C, N), F32, kind="ExternalInput").ap()
    g_dram = nc.dram_tensor("gamma", (1,), F32, kind="ExternalInput").ap()
    y_dram = nc.dram_tensor("y", (2, C, N), F32, kind="ExternalOutput").ap()

    with tile.TileContext(nc) as tc, ExitStack() as ctx:
        const = ctx.enter_context(tc.tile_pool(name="const", bufs=1))
        small = ctx.enter_context(tc.tile_pool(name="small", bufs=10))
        stage = ctx.enter_context(tc.tile_pool(name="stage", bufs=7))
        smx = ctx.enter_context(tc.tile_pool(name="smx", bufs=4))
        sqf = ctx.enter_context(tc.tile_pool(name="sqf", bufs=1))
        sqb = ctx.enter_context(tc.tile_pool(name="sqb", bufs=1))

        ident32 = const.tile([128, 128], F32, tag="id32")
        make_identity(nc, ident32[:])
        ident16 = const.tile([128, 128], F16, tag="id16")
        make_identity(nc, ident16[:])
        g_bc = const.tile([128, 1], F32, tag="gbc")
        nc.sync.dma_start(g_bc[:], g_dram[None, :].partition_broadcast(128))

        # persistent [512,512] matrices as [128, 4*512] (row-chunk r at cols r*512)
        er_sb = sqf.tile([128, MC * C], F32, tag="er")
        m12_sb = sqf.tile([128, MC * C], F32, tag="m12")
        s_sb = sqf.tile([128, MC * C], F32, tag="s")
        ei_sb = sqf.tile([128, MC * C], F32, tag="ei")
        att_sb = sqb.tile([128, MC * C], F16, tag="att")
        attT_sb = sqb.tile([128, MC * C], F16, tag="attT")

        with tc.tile_pool(name="opsT", bufs=4) as opsT:
            # transposed fp16 hi/lo operands, [128, NK*512]; chunk k at cols k*512
            arT = opsT.tile([128, NK * C], F16, tag="opsT")
            brT = opsT.tile([128, NK * C], F16, tag="opsT")
            aiT = opsT.tile([128, NK * C], F16, tag="opsT")
            biT = opsT.tile([128, NK * C], F16, tag="opsT")
            tens = [(arT, brT), (aiT, biT)]

            with tc.tile_pool(name="acc", bufs=6, space="PSUM") as acc:
                accs = {m: None for m in range(MC)}
                accs[0] = [acc.tile([128, C], F32, tag="acc", name=f"acc0_{i}")
                           for i in range(5)]

                # ------- Phase 0 + first m-pass, interleaved by quarter -------
                with tc.tile_pool(name="tpose", bufs=2, space="PSUM") as tpose:
                    for Q in range(NQ):
                        qlo = Q * KQ * 128          # column offset in x
                        for ti, (aT, bT) in enumerate(tens):
                            for j in range(MC):     # c-chunk (rows)
                                q_t = stage.tile([128, KQ * 128], F32, tag="stage")
                                nc.sync.dma_start(
                                    q_t[:], x_dram[ti, j * 128:(j + 1) * 128,
                                                   qlo:qlo + KQ * 128])
                                for kq in range(KQ // 4):  # psum groups of 4
                                    pt = tpose.tile([128, 512], F32, tag="pt")
                                    for t in range(4):
                                        kk = kq * 4 + t
                                        nc.tensor.transpose(
                                            pt[:, t * 128:(t + 1) * 128],
                                            q_t[:, kk * 128:(kk + 1) * 128],
                                            ident32[:])
                                    k0 = Q * KQ + kq * 4
                                    aT_v = aT[:].rearrange("p (k c) -> p k c", c=C)[
                                        :, k0:k0 + 4, j * 128:(j + 1) * 128]
                                    bT_v = bT[:].rearrange("p (k c) -> p k c", c=C)[
                                        :, k0:k0 + 4, j * 128:(j + 1) * 128]
                                    pt_v = pt[:].rearrange("p (t c) -> p t c", c=128)
                                    nc.scalar.copy(aT_v, pt_v)             # f32->f16
                                    nc.vector.tensor_sub(bT_v, pt_v, aT_v)  # lo part
                        # m=0 Gram MMs for this quarter's chunks
                        for k in range(Q * KQ, (Q + 1) * KQ):
                            _gram_mms(nc, accs[0], arT, brT, aiT, biT, 0, k)

                # ------- m-passes + evacuation + incremental symmetrize -------
                with tc.tile_pool(name="symt", bufs=2, space="PSUM") as symt:
                    def evac_and_sym(a):
                        u1, u2, m1, m2, s = accs[a]
                        er_a = er_sb[:, a * C:(a + 1) * C]
                        nc.scalar.copy(er_a, u1[:])
                        nc.vector.tensor_sub(er_a, er_a, u2[:])
                        m12_a = m12_sb[:, a * C:(a + 1) * C]
                        nc.scalar.copy(m12_a, m1[:])
                        nc.vector.tensor_sub(m12_a, m12_a, m2[:])
                        nc.scalar.copy(s_sb[:, a * C:(a + 1) * C], s[:])
                        nc.vector.tensor_add(er_a, er_a, m12_a)  # += M12 row a
                        # blocks (R, Cb) with max(R, Cb) == a are now computable
                        pairs = [(a, cb) for cb in range(a + 1)] + \
                                [(r, a) for r in range(a)]
                        # er(R, Cb) += T(M12(Cb, R))
                        for (R, Cb) in pairs:
                            pt = symt.tile([128, 128], F32, tag="symt",
                                           name=f"symt_er_{R}_{Cb}")
                            nc.tensor.transpose(
                                pt[:],
                                m12_sb[:, Cb * C + R * 128: Cb * C + (R + 1) * 128],
                                ident32[:])
                            dst = er_sb[:, R * C + Cb * 128: R * C + (Cb + 1) * 128]
                            nc.vector.tensor_add(dst, dst, pt[:])
                        # ei(R, Cb) = S(R, Cb) + T(S(Cb, R))
                        for (R, Cb) in pairs:
                            pt = symt.tile([128, 128], F32, tag="symt",
                                           name=f"symt_ei_{R}_{Cb}")
                            nc.tensor.transpose(
                                pt[:],
                                s_sb[:, Cb * C + R * 128: Cb * C + (R + 1) * 128],
                                ident32[:])
                            src = s_sb[:, R * C + Cb * 128: R * C + (Cb + 1) * 128]
                            dst = ei_sb[:, R * C + Cb * 128: R * C + (Cb + 1) * 128]
                            nc.vector.tensor_add(dst, src, pt[:])

                    evac_and_sym(0)
                    for m in range(1, MC):
                        accs[m] = [acc.tile([128, C], F32, tag="acc",
                                            name=f"acc{m}_{i}") for i in range(5)]
                        for k in range(NK):
                            _gram_mms(nc, accs[m], arT, brT, aiT, biT, m, k)
                        evac_and_sym(m)

        # ------------- tail (softmax, attT) + Phase 2, overlapped -----------
        with tc.tile_pool(name="slab32", bufs=6) as slab32, \
             tc.tile_pool(name="slab16", bufs=6) as slab16, \
             tc.tile_pool(name="ysb", bufs=8) as ysbp, \
             tc.tile_pool(name="attp", bufs=2, space="PSUM") as attp, \
             tc.tile_pool(name="out", bufs=6, space="PSUM") as outp:

            # emit ALL phase-2 slab loads + f16 casts up front; pool slot
            # rotation (bufs=6 -> 3 column-blocks in flight) throttles DMA.
            slabs, slabs_h = {}, {}
            for j in range(NJ):
                for ti in range(2):
                    sl = slab32.tile([128, MC, 512], F32, tag="sl32",
                                     name=f"sl_{j}_{ti}")
                    sh = slab16.tile([128, MC, 512], F16, tag="sl16",
                                     name=f"sh_{j}_{ti}")
                    for d in range(MC):
                        nc.sync.dma_start(
                            sl[:, d, :],
                            x_dram[ti, d * 128:(d + 1) * 128, j * 512:(j + 1) * 512])
                        if ti == 0:
                            nc.scalar.copy(sh[:, d, :], sl[:, d, :])
                        else:
                            nc.vector.tensor_copy(sh[:, d, :], sl[:, d, :])
                    slabs[(j, ti)] = sl
                    slabs_h[(j, ti)] = sh

            # ---------------- softmax over squared magnitude ----------------
            for m in range(MC):
                er_m = er_sb[:, m * C:(m + 1) * C]
                ei_m = ei_sb[:, m * C:(m + 1) * C]
                nmx_r = small.tile([128, 1], F32, tag="small")
                nmx_i = small.tile([128, 1], F32, tag="small")
                nc.vector.reduce_max(nmx_r[:], er_m, axis=AX.X, negate=True)
                nc.vector.reduce_max(nmx_i[:], ei_m, axis=AX.X, negate=True)
                sq1 = smx.tile([128, C], F32, tag="smx")
                nc.scalar.activation(sq1[:], er_m, AF.Square, bias=nmx_r[:, 0:1])
                sq2 = smx.tile([128, C], F32, tag="smx")
                nc.scalar.activation(sq2[:], ei_m, AF.Square, bias=nmx_i[:, 0:1])
                nc.vector.tensor_add(sq1[:], sq1[:], sq2[:])  # F
                nfm = small.tile([128, 1], F32, tag="small")
                nc.vector.reduce_max(nfm[:], sq1[:], axis=AX.X, negate=True)
                rsum = small.tile([128, 1], F32, tag="small")
                nc.scalar.activation(sq2[:], sq1[:], AF.Exp,
                                     bias=nfm[:, 0:1], accum_out=rsum[:, 0:1])
                rinv = small.tile([128, 1], F32, tag="small")
                nc.vector.reciprocal(rinv[:], rsum[:])
                nc.vector.tensor_scalar_mul(
                    att_sb[:, m * C:(m + 1) * C], sq2[:], rinv[:, 0:1])

            # ---------------- attT (fp16 PE transposes) ---------------------
            for d in range(MC):
                pt = attp.tile([128, C], F16, tag="attt")
                for m in range(MC):
                    nc.tensor.transpose(
                        pt[:, m * 128:(m + 1) * 128],
                        att_sb[:, m * C + d * 128: m * C + (d + 1) * 128],
                        ident16[:])
                nc.scalar.copy(attT_sb[:, d * C:(d + 1) * C], pt[:])

            # ---------------- Phase 2 compute: gamma*(att@q) + x ------------
            for j in range(NJ):
                for ti in range(2):
                    for m in range(MC):
                        ops = outp.tile([128, 512], F32, tag="out")
                        for d in range(MC):
                            nc.tensor.matmul(
                                ops[:],
                                attT_sb[:, d * C + m * 128: d * C + (m + 1) * 128],
                                slabs_h[(j, ti)][:, d, :],
                                start=(d == 0), stop=(d == MC - 1))
                        y_t = ysbp.tile([128, 512], F32, tag="ysb")
                        nc.vector.scalar_tensor_tensor(
                            y_t[:], ops[:], g_bc[:, 0:1], slabs[(j, ti)][:, m, :],
                            op0=OP.mult, op1=OP.add)
                        nc.sync.dma_start(
                            y_dram[ti, m * 128:(m + 1) * 128, j * 512:(j + 1) * 512],
                            y_t[:])

    nc.compile()
    return nc


_NC_CACHE = None


def kernel(x: np.ndarray, gamma: np.ndarray) -> np.ndarray:
    global _NC_CACHE
    if _NC_CACHE is None:
        _NC_CACHE = build_kernel()
    nc = _NC_CACHE
    B = x.shape[1]
    x = np.ascontiguousarray(x, dtype=np.float32)
    in_maps = [{"x": np.ascontiguousarray(x[:, b]).reshape(2, C, N),
                "gamma": np.ascontiguousarray(gamma, dtype=np.float32)}
               for b in range(B)]
    res = run_bass_kernel_spmd(nc, in_maps, core_ids=list(range(B)))
    y = np.stack([res.results[b]["y"] for b in range(B)], axis=1)
    return y.reshape(x.shape)


# revision 11
# speedup vs baseline: 187.8750x; 2.5034x over previous
"""CAM-module (complex channel-attention) Bass kernel for Trainium2.

Problem: x (2, 8, 512, 4, 32, 32) fp32 -> out same shape.
  qr, qi = x[0].reshape(B,C,N), x[1].reshape(B,C,N)   C=512, N=4096
  er = qr qr^T - qi qi^T ; ei = qr qi^T + qi qr^T     (B, C, C)
  F  = (rowmax(er)-er)^2 + (rowmax(ei)-ei)^2
  att = softmax_row(F)                                 (ultra-sharp)
  out = stack(g*att@qr + qr_in, g*att@qi + qi_in)

Sharding: data-parallel over batch B across 8 NeuronCores (core b = batch b).

Numerics: the softmax is near-one-hot with top-2 F-gaps as small as ~18, so
the Gram phase needs fp32-quality precision; single-dtype bf16/tf32 matmuls
flip argmax rows. We use an fp16 hi/lo split (A = f16(q), Bl = f16(q - A))
and compute
  qq'^T ~= A A'^T + A Bl'^T + Bl A'^T   (dropping Bl Bl'^T, err ~1e-5)
at full 1-cycle/row PE rate. Symmetric cross terms are computed once and
symmetrized via PE transposes:
  er = AA_r - AA_i + M12 + M12^T,  M12 = A_r Bl_r^T - A_i Bl_i^T
  ei = S + S^T,                    S   = A_r A_i^T + A_r Bl_i^T + Bl_r A_i^T
Phase 2 (att @ q) runs in fp16 (error ~1e-5 of O(1) outputs).

Schedule notes:
 - input loaded in column-quarters so the first Gram pass interleaves with
   the PE input transposes;
 - symmetrization transpose-adds are emitted incrementally after each
   m-pass evacuation (only the last pass's blocks land in the tail);
 - phase-2 slab DMA is emitted before the softmax tail so it overlaps.
"""
import sys, os
sys.path.insert(0, '/opt/trn_rl_repo')

import numpy as np
from contextlib import ExitStack

import concourse.bass as bass
import concourse.mybir as mybir
import concourse.tile as tile
from concourse import bacc
from concourse.bass_utils import run_bass_kernel_spmd
from concourse.masks import make_identity

F32 = mybir.dt.float32
F16 = mybir.dt.float16
AX = mybir.AxisListType
AF = mybir.ActivationFunctionType
OP = mybir.AluOpType

C = 512          # channels
N = 4096         # spatial (4*32*32)
NK = N // 128    # 32 n-chunks (contraction)
MC = C // 128    # 4 c-chunks
NQ = 4           # column quarters for load/transpose pipeline
KQ = NK // NQ    # 8 n-chunks per quarter
NJ = N // 512    # phase-2 column blocks


def _gram_mms(nc, accs, arT, brT, aiT, biT, m, k):
    """The 7 phase-1 matmuls for (m-block, k-chunk), grouped by lhsT."""
    u1, u2, m1, m2, s = accs
    st, sp = (k == 0), (k == NK - 1)
    lo = k * C
    ar_m = arT[:, lo + m * 128: lo + (m + 1) * 128]
    ai_m = aiT[:, lo + m * 128: lo + (m + 1) * 128]
    br_m = brT[:, lo + m * 128: lo + (m + 1) * 128]
    ar_k = arT[:, lo:lo + C]
    ai_k = aiT[:, lo:lo + C]
    br_k = brT[:, lo:lo + C]
    bi_k = biT[:, lo:lo + C]
    # lhsT = ar_m
    nc.tensor.matmul(u1[:], ar_m, ar_k, start=st, stop=sp)
    nc.tensor.matmul(m1[:], ar_m, br_k, start=st, stop=sp)
    nc.tensor.matmul(s[:], ar_m, ai_k, start=st, stop=False)
    nc.tensor.matmul(s[:], ar_m, bi_k, start=False, stop=False)
    # lhsT = ai_m
    nc.tensor.matmul(u2[:], ai_m, ai_k, start=st, stop=sp)
    nc.tensor.matmul(m2[:], ai_m, bi_k, start=st, stop=sp)
    # lhsT = br_m
    nc.tensor.matmul(s[:], br_m, ai_k, start=False, stop=sp)


def build_kernel():
    nc = bacc.Bacc("TRN2", target_bir_lowering=False, debug=False,
                   enable_asserts=False)
    x_dram = nc.dram_tensor("x", (2, C, N), F32, kind="ExternalInput").ap()
    g_dram = nc.dram_tensor("gamma", (1,), F32, kind="ExternalInput").ap()
    y_dram = nc.dram_tensor("y", (2, C, N), F32, kind="ExternalOutput").ap()

    with tile.TileContext(nc) as tc, ExitStack() as ctx:
        const = ctx.enter_context(tc.tile_pool(name="const", bufs=1))
        small = ctx.enter_context(tc.tile_pool(name="small", bufs=10))
        stage = ctx.enter_context(tc.tile_pool(name="stage", bufs=7))
        smx = ctx.enter_context(tc.tile_pool(name="smx", bufs=4))
        sqf = ctx.enter_context(tc.tile_pool(name="sqf", bufs=1))
        sqb = ctx.enter_context(tc.tile_pool(name="sqb", bufs=1))

        ident32 = const.tile([128, 128], F32, tag="id32")
        make_identity(nc, ident32[:])
        ident16 = const.tile([128, 128], F16, tag="id16")
        make_identity(nc, ident16[:])
        g_bc = const.tile([128, 1], F32, tag="gbc")
        nc.sync.dma_start(g_bc[:], g_dram[None, :].partition_broadcast(128))

        # persistent [512,512] matrices as [128, 4*512] (row-chunk r at cols r*512)
        er_sb = sqf.tile([128, MC * C], F32, tag="er")
        m12_sb = sqf.tile([128, MC * C], F32, tag="m12")
        s_sb = sqf.tile([128, MC * C], F32, tag="s")
        ei_sb = sqf.tile([128, MC * C], F32, tag="ei")
        att_sb = sqb.tile([128, MC * C], F16, tag="att")
        attT_sb = sqb.tile([128, MC * C], F16, tag="attT")

        with tc.tile_pool(name="opsT", bufs=4) as opsT:
            # transposed fp16 hi/lo operands, [128, NK*512]; chunk k at cols k*512
            arT = opsT.tile([128, NK * C], F16, tag="opsT")
            brT = opsT.tile([128, NK * C], F16, tag="opsT")
            aiT = opsT.tile([128, NK * C], F16, tag="opsT")
            biT = opsT.tile([128, NK * C], F16, tag="opsT")
            tens = [(arT, brT), (aiT, biT)]

            with tc.tile_pool(name="acc", bufs=6, space="PSUM") as acc:
                accs = {m: None for m in range(MC)}
                accs[0] = [acc.tile([128, C], F32, tag="acc", name=f"acc0_{i}")
                           for i in range(5)]

                # ------- Phase 0 + first m-pass, interleaved by quarter -------
                with tc.tile_pool(name="tpose", bufs=2, space="PSUM") as tpose:
                    for Q in range(NQ):
                        qlo = Q * KQ * 128          # column offset in x
                        for ti, (aT, bT) in enumerate(tens):
                            for j in range(MC):     # c-chunk (rows)
                                q_t = stage.tile([128, KQ * 128], F32, tag="stage")
                                nc.sync.dma_start(
                                    q_t[:], x_dram[ti, j * 128:(j + 1) * 128,
                                                   qlo:qlo + KQ * 128])
                                for kq in range(KQ // 4):  # psum groups of 4
                                    pt = tpose.tile([128, 512], F32, tag="pt")
                                    for t in range(4):
                                        kk = kq * 4 + t
                                        nc.tensor.transpose(
                                            pt[:, t * 128:(t + 1) * 128],
                                            q_t[:, kk * 128:(kk + 1) * 128],
                                            ident32[:])
                                    k0 = Q * KQ + kq * 4
                                    aT_v = aT[:].rearrange("p (k c) -> p k c", c=C)[
                                        :, k0:k0 + 4, j * 128:(j + 1) * 128]
                                    bT_v = bT[:].rearrange("p (k c) -> p k c", c=C)[
                                        :, k0:k0 + 4, j * 128:(j + 1) * 128]
                                    pt_v = pt[:].rearrange("p (t c) -> p t c", c=128)
                                    nc.scalar.copy(aT_v, pt_v)             # f32->f16
                                    nc.vector.tensor_sub(bT_v, pt_v, aT_v)  # lo part
                        # m=0 Gram MMs for this quarter's chunks
                        for k in range(Q * KQ, (Q + 1) * KQ):
                            _gram_mms(nc, accs[0], arT, brT, aiT, biT, 0, k)

                # ------- m-passes + evacuation + incremental symmetrize -------
                with tc.tile_pool(name="symt", bufs=2, space="PSUM") as symt:
                    def evac_and_sym(a):
                        u1, u2, m1, m2, s = accs[a]
                        er_a = er_sb[:, a * C:(a + 1) * C]
                        nc.scalar.copy(er_a, u1[:])
                        nc.vector.tensor_sub(er_a, er_a, u2[:])
                        m12_a = m12_sb[:, a * C:(a + 1) * C]
                        nc.scalar.copy(m12_a, m1[:])
                        nc.vector.tensor_sub(m12_a, m12_a, m2[:])
                        nc.scalar.copy(s_sb[:, a * C:(a + 1) * C], s[:])
                        nc.vector.tensor_add(er_a, er_a, m12_a)  # += M12 row a
                        # blocks (R, Cb) with max(R, Cb) == a are now computable
                        pairs = [(a, cb) for cb in range(a + 1)] + \
                                [(r, a) for r in range(a)]
                        # er(R, Cb) += T(M12(Cb, R))
                        for (R, Cb) in pairs:
                            pt = symt.tile([128, 128], F32, tag="symt",
                                           name=f"symt_er_{R}_{Cb}")
                            nc.tensor.transpose(
                                pt[:],
                                m12_sb[:, Cb * C + R * 128: Cb * C + (R + 1) * 128],
                                ident32[:])
                            dst = er_sb[:, R * C + Cb * 128: R * C + (Cb + 1) * 128]
                            nc.vector.tensor_add(dst, dst, pt[:])
                        # ei(R, Cb) = S(R, Cb) + T(S(Cb, R))
                        for (R, Cb) in pairs:
                            pt = symt.tile([128, 128], F32, tag="symt",
                                           name=f"symt_ei_{R}_{Cb}")
                            nc.tensor.transpose(
                                pt[:],
                                s_sb[:, Cb * C + R * 128: Cb * C + (R + 1) * 128],
                                ident32[:])
                            src = s_sb[:, R * C + Cb * 128: R * C + (Cb + 1) * 128]
                            dst = ei_sb[:, R * C + Cb * 128: R * C + (Cb + 1) * 128]
                            nc.vector.tensor_add(dst, src, pt[:])

                    evac_and_sym(0)
                    for m in range(1, MC):
                        accs[m] = [acc.tile([128, C], F32, tag="acc",
                                            name=f"acc{m}_{i}") for i in range(5)]
                        for k in range(NK):
                            _gram_mms(nc, accs[m], arT, brT, aiT, biT, m, k)
                        evac_and_sym(m)

        # ------------- tail (softmax, attT) + Phase 2, overlapped -----------
        with tc.tile_pool(name="slab32", bufs=8) as slab32, \
             tc.tile_pool(name="slab16", bufs=8) as slab16, \
             tc.tile_pool(name="ysb", bufs=12) as ysbp, \
             tc.tile_pool(name="attp", bufs=2, space="PSUM") as attp, \
             tc.tile_pool(name="out", bufs=6, space="PSUM") as outp:

            # emit ALL phase-2 slab loads + f16 casts up front; pool slot
            # rotation (bufs=6 -> 3 column-blocks in flight) throttles DMA.
            slabs, slabs_h = {}, {}
            for j in range(NJ):
                for ti in range(2):
                    sl = slab32.tile([128, MC, 512], F32, tag="sl32",
                                     name=f"sl_{j}_{ti}")
                    sh = slab16.tile([128, MC, 512], F16, tag="sl16",
                                     name=f"sh_{j}_{ti}")
                    for d in range(MC):
                        nc.sync.dma_start(
                            sl[:, d, :],
                            x_dram[ti, d * 128:(d + 1) * 128, j * 512:(j + 1) * 512])
                        if ti == 0:
                            nc.scalar.copy(sh[:, d, :], sl[:, d, :])
                        else:
                            nc.vector.tensor_copy(sh[:, d, :], sl[:, d, :])
                    slabs[(j, ti)] = sl
                    slabs_h[(j, ti)] = sh

            # ---------------- softmax over squared magnitude ----------------
            for m in range(MC):
                er_m = er_sb[:, m * C:(m + 1) * C]
                ei_m = ei_sb[:, m * C:(m + 1) * C]
                nmx_r = small.tile([128, 1], F32, tag="small")
                nmx_i = small.tile([128, 1], F32, tag="small")
                nc.vector.reduce_max(nmx_r[:], er_m, axis=AX.X, negate=True)
                nc.vector.reduce_max(nmx_i[:], ei_m, axis=AX.X, negate=True)
                sq1 = smx.tile([128, C], F32, tag="smx")
                nc.scalar.activation(sq1[:], er_m, AF.Square, bias=nmx_r[:, 0:1])
                sq2 = smx.tile([128, C], F32, tag="smx")
                nc.scalar.activation(sq2[:], ei_m, AF.Square, bias=nmx_i[:, 0:1])
                nc.vector.tensor_add(sq1[:], sq1[:], sq2[:])  # F
                nfm = small.tile([128, 1], F32, tag="small")
                nc.vector.reduce_max(nfm[:], sq1[:], axis=AX.X, negate=True)
                rsum = small.tile([128, 1], F32, tag="small")
                nc.scalar.activation(sq2[:], sq1[:], AF.Exp,
                                     bias=nfm[:, 0:1], accum_out=rsum[:, 0:1])
                rinv = small.tile([128, 1], F32, tag="small")
                nc.vector.reciprocal(rinv[:], rsum[:])
                nc.vector.tensor_scalar_mul(
                    att_sb[:, m * C:(m + 1) * C], sq2[:], rinv[:, 0:1])

            # ---------------- attT (fp16 PE transposes) ---------------------
            for d in range(MC):
                pt = attp.tile([128, C], F16, tag="attt")
                for m in range(MC):
                    nc.tensor.transpose(
                        pt[:, m * 128:(m + 1) * 128],
                        att_sb[:, m * C + d * 128: m * C + (d + 1) * 128],
                        ident16[:])
                nc.scalar.copy(attT_sb[:, d * C:(d + 1) * C], pt[:])

            # ---------------- Phase 2 compute: gamma*(att@q) + x ------------
            for j in range(NJ):
                for ti in range(2):
                    for m in range(MC):
                        ops = outp.tile([128, 512], F32, tag="out")
                        for d in range(MC):
                            nc.tensor.matmul(
                                ops[:],
                                attT_sb[:, d * C + m * 128: d * C + (m + 1) * 128],
                                slabs_h[(j, ti)][:, d, :],
                                start=(d == 0), stop=(d == MC - 1))
                        y_t = ysbp.tile([128, 512], F32, tag="ysb")
                        nc.vector.scalar_tensor_tensor(
                            y_t[:], ops[:], g_bc[:, 0:1], slabs[(j, ti)][:, m, :],
                            op0=OP.mult, op1=OP.add)
                        nc.sync.dma_start(
                            y_dram[ti, m * 128:(m + 1) * 128, j * 512:(j + 1) * 512],
                            y_t[:])

    nc.compile()
    return nc


_NC_CACHE = None


def kernel(x: np.ndarray, gamma: np.ndarray) -> np.ndarray:
    global _NC_CACHE
    if _NC_CACHE is None:
        _NC_CACHE = build_kernel()
    nc = _NC_CACHE
    B = x.shape[1]
    x = np.ascontiguousarray(x, dtype=np.float32)
    in_maps = [{"x": np.ascontiguousarray(x[:, b]).reshape(2, C, N),
                "gamma": np.ascontiguousarray(gamma, dtype=np.float32)}
               for b in range(B)]
    res = run_bass_kernel_spmd(nc, in_maps, core_ids=list(range(B)))
    y = np.stack([res.results[b]["y"] for b in range(B)], axis=1)
    return y.reshape(x.shape)


# revision 13
# speedup vs baseline: 188.0590x; 1.0010x over previous
"""CAM-module (complex channel-attention) Bass kernel for Trainium2.

Problem: x (2, 8, 512, 4, 32, 32) fp32 -> out same shape.
  qr, qi = x[0].reshape(B,C,N), x[1].reshape(B,C,N)   C=512, N=4096
  er = qr qr^T - qi qi^T ; ei = qr qi^T + qi qr^T     (B, C, C)
  F  = (rowmax(er)-er)^2 + (rowmax(ei)-ei)^2
  att = softmax_row(F)                                 (ultra-sharp)
  out = stack(g*att@qr + qr_in, g*att@qi + qi_in)

Sharding: data-parallel over batch B across 8 NeuronCores (core b = batch b).

Numerics: the softmax is near-one-hot with top-2 F-gaps as small as ~18, so
the Gram phase needs fp32-quality precision; single-dtype bf16/tf32 matmuls
flip argmax rows. We use an fp16 hi/lo split (A = f16(q), Bl = f16(q - A))
and compute
  qq'^T ~= A A'^T + A Bl'^T + Bl A'^T   (dropping Bl Bl'^T, err ~1e-5)
at full 1-cycle/row PE rate. Symmetric cross terms are computed once and
symmetrized via PE transposes:
  er = AA_r - AA_i + M12 + M12^T,  M12 = A_r Bl_r^T - A_i Bl_i^T
  ei = S + S^T,                    S   = A_r A_i^T + A_r Bl_i^T + Bl_r A_i^T
Phase 2 (att @ q) runs in fp16 (error ~1e-5 of O(1) outputs).

Schedule notes:
 - input loaded in column-quarters so the first Gram pass interleaves with
   the PE input transposes;
 - symmetrization transpose-adds are emitted incrementally after each
   m-pass evacuation (only the last pass's blocks land in the tail);
 - phase-2 slab DMA is emitted before the softmax tail so it overlaps.
"""
import sys, os
sys.path.insert(0, '/opt/trn_rl_repo')

import numpy as np
from contextlib import ExitStack

import concourse.bass as bass
import concourse.mybir as mybir
import concourse.tile as tile
from concourse import bacc
from concourse.bass_utils import run_bass_kernel_spmd
from concourse.masks import make_identity

F32 = mybir.dt.float32
F16 = mybir.dt.float16
AX = mybir.AxisListType
AF = mybir.ActivationFunctionType
OP = mybir.AluOpType

C = 512          # channels
N = 4096         # spatial (4*32*32)
NK = N // 128    # 32 n-chunks (contraction)
MC = C // 128    # 4 c-chunks
NQ = 4           # column quarters for load/transpose pipeline
KQ = NK // NQ    # 8 n-chunks per quarter
NJ = N // 512    # phase-2 column blocks


def _gram_mms(nc, accs, arT, brT, aiT, biT, m, k):
    """The 7 phase-1 matmuls for (m-block, k-chunk), grouped by lhsT."""
    u1, u2, m1, m2, s = accs
    st, sp = (k == 0), (k == NK - 1)
    lo = k * C
    ar_m = arT[:, lo + m * 128: lo + (m + 1) * 128]
    ai_m = aiT[:, lo + m * 128: lo + (m + 1) * 128]
    br_m = brT[:, lo + m * 128: lo + (m + 1) * 128]
    ar_k = arT[:, lo:lo + C]
    ai_k = aiT[:, lo:lo + C]
    br_k = brT[:, lo:lo + C]
    bi_k = biT[:, lo:lo + C]
    # lhsT = ar_m
    nc.tensor.matmul(u1[:], ar_m, ar_k, start=st, stop=sp)
    nc.tensor.matmul(m1[:], ar_m, br_k, start=st, stop=sp)
    nc.tensor.matmul(s[:], ar_m, ai_k, start=st, stop=False)
    nc.tensor.matmul(s[:], ar_m, bi_k, start=False, stop=False)
    # lhsT = ai_m
    nc.tensor.matmul(u2[:], ai_m, ai_k, start=st, stop=sp)
    nc.tensor.matmul(m2[:], ai_m, bi_k, start=st, stop=sp)
    # lhsT = br_m
    nc.tensor.matmul(s[:], br_m, ai_k, start=False, stop=sp)


def build_kernel():
    nc = bacc.Bacc("TRN2", target_bir_lowering=False, debug=False,
                   enable_asserts=False)
    x_dram = nc.dram_tensor("x", (2, C, N), F32, kind="ExternalInput").ap()
    g_dram = nc.dram_tensor("gamma", (1,), F32, kind="ExternalInput").ap()
    y_dram = nc.dram_tensor("y", (2, C, N), F32, kind="ExternalOutput").ap()

    with tile.TileContext(nc) as tc, ExitStack() as ctx:
        const = ctx.enter_context(tc.tile_pool(name="const", bufs=1))
        small = ctx.enter_context(tc.tile_pool(name="small", bufs=10))
        stage = ctx.enter_context(tc.tile_pool(name="stage", bufs=14))
        smx = ctx.enter_context(tc.tile_pool(name="smx", bufs=4))
        sqf = ctx.enter_context(tc.tile_pool(name="sqf", bufs=1))
        sqb = ctx.enter_context(tc.tile_pool(name="sqb", bufs=1))

        ident32 = const.tile([128, 128], F32, tag="id32")
        make_identity(nc, ident32[:])
        ident16 = const.tile([128, 128], F16, tag="id16")
        make_identity(nc, ident16[:])
        g_bc = const.tile([128, 1], F32, tag="gbc")
        nc.sync.dma_start(g_bc[:], g_dram[None, :].partition_broadcast(128))

        # persistent [512,512] matrices as [128, 4*512] (row-chunk r at cols r*512)
        er_sb = sqf.tile([128, MC * C], F32, tag="er")
        m12_sb = sqf.tile([128, MC * C], F32, tag="m12")
        s_sb = sqf.tile([128, MC * C], F32, tag="s")
        ei_sb = sqf.tile([128, MC * C], F32, tag="ei")
        att_sb = sqb.tile([128, MC * C], F16, tag="att")
        attT_sb = sqb.tile([128, MC * C], F16, tag="attT")

        with tc.tile_pool(name="opsT", bufs=4) as opsT:
            # transposed fp16 hi/lo operands, [128, NK*512]; chunk k at cols k*512
            arT = opsT.tile([128, NK * C], F16, tag="opsT")
            brT = opsT.tile([128, NK * C], F16, tag="opsT")
            aiT = opsT.tile([128, NK * C], F16, tag="opsT")
            biT = opsT.tile([128, NK * C], F16, tag="opsT")
            tens = [(arT, brT), (aiT, biT)]

            with tc.tile_pool(name="acc", bufs=6, space="PSUM") as acc:
                accs = {m: None for m in range(MC)}
                accs[0] = [acc.tile([128, C], F32, tag="acc", name=f"acc0_{i}")
                           for i in range(5)]

                # ------- Phase 0 + first m-pass, interleaved by quarter -------
                with tc.tile_pool(name="tpose", bufs=2, space="PSUM") as tpose:
                    for Q in range(NQ):
                        for ti, (aT, bT) in enumerate(tens):
                            for j in range(MC):     # c-chunk (rows)
                                for kq in range(KQ // 4):  # 4-chunk groups
                                    k0 = Q * KQ + kq * 4
                                    q_t = stage.tile([128, 512], F32, tag="stage")
                                    nc.sync.dma_start(
                                        q_t[:], x_dram[ti, j * 128:(j + 1) * 128,
                                                       k0 * 128:(k0 + 4) * 128])
                                    pt = tpose.tile([128, 512], F32, tag="pt")
                                    for t in range(4):
                                        nc.tensor.transpose(
                                            pt[:, t * 128:(t + 1) * 128],
                                            q_t[:, t * 128:(t + 1) * 128],
                                            ident32[:])
                                    aT_v = aT[:].rearrange("p (k c) -> p k c", c=C)[
                                        :, k0:k0 + 4, j * 128:(j + 1) * 128]
                                    bT_v = bT[:].rearrange("p (k c) -> p k c", c=C)[
                                        :, k0:k0 + 4, j * 128:(j + 1) * 128]
                                    pt_v = pt[:].rearrange("p (t c) -> p t c", c=128)
                                    nc.scalar.copy(aT_v, pt_v)             # f32->f16
                                    nc.vector.tensor_sub(bT_v, pt_v, aT_v)  # lo part
                        # m=0 Gram MMs for this quarter's chunks
                        for k in range(Q * KQ, (Q + 1) * KQ):
                            _gram_mms(nc, accs[0], arT, brT, aiT, biT, 0, k)

                # ------- m-passes + evacuation + incremental symmetrize -------
                with tc.tile_pool(name="symt", bufs=2, space="PSUM") as symt:
                    def evac_and_sym(a):
                        u1, u2, m1, m2, s = accs[a]
                        er_a = er_sb[:, a * C:(a + 1) * C]
                        nc.scalar.copy(er_a, u1[:])
                        nc.vector.tensor_sub(er_a, er_a, u2[:])
                        m12_a = m12_sb[:, a * C:(a + 1) * C]
                        nc.scalar.copy(m12_a, m1[:])
                        nc.vector.tensor_sub(m12_a, m12_a, m2[:])
                        nc.scalar.copy(s_sb[:, a * C:(a + 1) * C], s[:])
                        nc.vector.tensor_add(er_a, er_a, m12_a)  # += M12 row a
                        # blocks (R, Cb) with max(R, Cb) == a are now computable
                        pairs = [(a, cb) for cb in range(a + 1)] + \
                                [(r, a) for r in range(a)]
                        # er(R, Cb) += T(M12(Cb, R))
                        for (R, Cb) in pairs:
                            pt = symt.tile([128, 128], F32, tag="symt",
                                           name=f"symt_er_{R}_{Cb}")
                            nc.tensor.transpose(
                                pt[:],
                                m12_sb[:, Cb * C + R * 128: Cb * C + (R + 1) * 128],
                                ident32[:])
                            dst = er_sb[:, R * C + Cb * 128: R * C + (Cb + 1) * 128]
                            nc.vector.tensor_add(dst, dst, pt[:])
                        # ei(R, Cb) = S(R, Cb) + T(S(Cb, R))
                        for (R, Cb) in pairs:
                            pt = symt.tile([128, 128], F32, tag="symt",
                                           name=f"symt_ei_{R}_{Cb}")
                            nc.tensor.transpose(
                                pt[:],
                                s_sb[:, Cb * C + R * 128: Cb * C + (R + 1) * 128],
                                ident32[:])
                            src = s_sb[:, R * C + Cb * 128: R * C + (Cb + 1) * 128]
                            dst = ei_sb[:, R * C + Cb * 128: R * C + (Cb + 1) * 128]
                            nc.vector.tensor_add(dst, src, pt[:])

                    evac_and_sym(0)
                    for m in range(1, MC):
                        accs[m] = [acc.tile([128, C], F32, tag="acc",
                                            name=f"acc{m}_{i}") for i in range(5)]
                        for k in range(NK):
                            _gram_mms(nc, accs[m], arT, brT, aiT, biT, m, k)
                        evac_and_sym(m)

        # ------------- tail (softmax, attT) + Phase 2, overlapped -----------
        with tc.tile_pool(name="slab32", bufs=8) as slab32, \
             tc.tile_pool(name="slab16", bufs=8) as slab16, \
             tc.tile_pool(name="ysb", bufs=12) as ysbp, \
             tc.tile_pool(name="attp", bufs=2, space="PSUM") as attp, \
             tc.tile_pool(name="out", bufs=6, space="PSUM") as outp:

            # emit ALL phase-2 slab loads + f16 casts up front; pool slot
            # rotation (bufs=6 -> 3 column-blocks in flight) throttles DMA.
            slabs, slabs_h = {}, {}
            for j in range(NJ):
                for ti in range(2):
                    sl = slab32.tile([128, MC, 512], F32, tag="sl32",
                                     name=f"sl_{j}_{ti}")
                    sh = slab16.tile([128, MC, 512], F16, tag="sl16",
                                     name=f"sh_{j}_{ti}")
                    for d in range(MC):
                        nc.sync.dma_start(
                            sl[:, d, :],
                            x_dram[ti, d * 128:(d + 1) * 128, j * 512:(j + 1) * 512])
                        if ti == 0:
                            nc.scalar.copy(sh[:, d, :], sl[:, d, :])
                        else:
                            nc.vector.tensor_copy(sh[:, d, :], sl[:, d, :])
                    slabs[(j, ti)] = sl
                    slabs_h[(j, ti)] = sh

            # ---------------- softmax over squared magnitude ----------------
            for m in range(MC):
                er_m = er_sb[:, m * C:(m + 1) * C]
                ei_m = ei_sb[:, m * C:(m + 1) * C]
                nmx_r = small.tile([128, 1], F32, tag="small")
                nmx_i = small.tile([128, 1], F32, tag="small")
                nc.vector.reduce_max(nmx_r[:], er_m, axis=AX.X, negate=True)
                nc.vector.reduce_max(nmx_i[:], ei_m, axis=AX.X, negate=True)
                sq1 = smx.tile([128, C], F32, tag="smx")
                nc.scalar.activation(sq1[:], er_m, AF.Square, bias=nmx_r[:, 0:1])
                sq2 = smx.tile([128, C], F32, tag="smx")
                nc.scalar.activation(sq2[:], ei_m, AF.Square, bias=nmx_i[:, 0:1])
                nc.vector.tensor_add(sq1[:], sq1[:], sq2[:])  # F
                nfm = small.tile([128, 1], F32, tag="small")
                nc.vector.reduce_max(nfm[:], sq1[:], axis=AX.X, negate=True)
                rsum = small.tile([128, 1], F32, tag="small")
                nc.scalar.activation(sq2[:], sq1[:], AF.Exp,
                                     bias=nfm[:, 0:1], accum_out=rsum[:, 0:1])
                rinv = small.tile([128, 1], F32, tag="small")
                nc.vector.reciprocal(rinv[:], rsum[:])
                nc.vector.tensor_scalar_mul(
                    att_sb[:, m * C:(m + 1) * C], sq2[:], rinv[:, 0:1])
                # attT for this m right away (m-major blocks: d at cols d*128)
                pt = attp.tile([128, C], F16, tag="attt")
                for d in range(MC):
                    nc.tensor.transpose(
                        pt[:, d * 128:(d + 1) * 128],
                        att_sb[:, m * C + d * 128: m * C + (d + 1) * 128],
                        ident16[:])
                nc.scalar.copy(attT_sb[:, m * C:(m + 1) * C], pt[:])

            # ---------------- Phase 2 compute: gamma*(att@q) + x ------------
            for j in range(NJ):
                for ti in range(2):
                    for m in range(MC):
                        ops = outp.tile([128, 512], F32, tag="out")
                        for d in range(MC):
                            nc.tensor.matmul(
                                ops[:],
                                attT_sb[:, m * C + d * 128: m * C + (d + 1) * 128],
                                slabs_h[(j, ti)][:, d, :],
                                start=(d == 0), stop=(d == MC - 1))
                        y_t = ysbp.tile([128, 512], F32, tag="ysb")
                        nc.vector.scalar_tensor_tensor(
                            y_t[:], ops[:], g_bc[:, 0:1], slabs[(j, ti)][:, m, :],
                            op0=OP.mult, op1=OP.add)
                        nc.sync.dma_start(
                            y_dram[ti, m * 128:(m + 1) * 128, j * 512:(j + 1) * 512],
                            y_t[:])

    nc.compile()
    return nc


_NC_CACHE = None


def kernel(x: np.ndarray, gamma: np.ndarray) -> np.ndarray:
    global _NC_CACHE
    if _NC_CACHE is None:
        _NC_CACHE = build_kernel()
    nc = _NC_CACHE
    B = x.shape[1]
    x = np.ascontiguousarray(x, dtype=np.float32)
    in_maps = [{"x": np.ascontiguousarray(x[:, b]).reshape(2, C, N),
                "gamma": np.ascontiguousarray(gamma, dtype=np.float32)}
               for b in range(B)]
    res = run_bass_kernel_spmd(nc, in_maps, core_ids=list(range(B)))
    y = np.stack([res.results[b]["y"] for b in range(B)], axis=1)
    return y.reshape(x.shape)
